# revision 1
# baseline (speedup 1.0000x reference)
"""Bass/Trainium2 kernel for nn_BaselineLSTM (B=2048, T=512, H=128, twin=256).

Strategy:
  - Data-parallel: batch 2048 -> 8 cores x 256; each core runs 2 interleaved
    chunks of 128 batch (pipelining hides per-step cross-engine latency).
  - State kept transposed: hT/cT = [H=128 partitions, batch free]. Gate
    matmuls are out[gate_rows, batch] = W_slice.T.T @ hT -> no per-step
    transpose anywhere.
  - Gates land in one PSUM bank per chunk-step ordered [i|f|o|g] so a single
    merged Sigmoid ACT covers i,f,o; Tanh covers g.
  - Phase P (teacher forcing): input + bias enter via a K=2 accumulating
    matmul against packed rows [y_t; 1].
  - Phase H (autoregressive): x_t = W_out h + b_out is folded into the
    recurrence:  g = (W_hh + W_ih W_out) h + (b + W_ih b_out). No feedback
    data path; bias enters via a K=1 matmul against a ones row.
  - h lives in a 4-slot SBUF ring; predictions p_t = W_out h_t are batched
    4 steps per matmul (shifted zero-padded stationary places each group in
    its own PSUM row), flushed to DRAM every 128 steps. b_out is added on
    the host.
  - The two chunks are emitted half a step out of phase (software pipeline);
    tanh(g) is issued before the i/f/o sigmoid so the c-update chain starts
    as early as possible. All matmul/elementwise data is bf16 (PSUM
    accumulation in f32); rel err vs the f32 reference is ~0.9% of absmax.
"""

import functools

import ml_dtypes
import numpy as np

import concourse.bacc as bacc
import concourse.tile as tile
from concourse import mybir
from concourse.bass_utils import run_bass_kernel_spmd

F32 = mybir.dt.float32
BF16 = mybir.dt.bfloat16
AF = mybir.ActivationFunctionType

H = 128          # hidden
NCORES = 8
BS = 256         # batch per core
BC = 128         # batch per chunk
NCHUNK = 2

# pytorch gate order (i, f, g, o) -> kernel order (i, f, o, g)
_PERM = np.concatenate([np.arange(0, 128), np.arange(128, 256),
                        np.arange(384, 512), np.arange(256, 384)])


def _build_body(tc, d, NP, NH, EPOCH):
    nc = tc.nc
    NT = NP + NH

    import contextlib
    with contextlib.ExitStack() as ctx:
        consts = ctx.enter_context(tc.tile_pool(name="consts", bufs=1))
        state = ctx.enter_context(tc.tile_pool(name="state", bufs=1))
        work = ctx.enter_context(tc.tile_pool(name="work", bufs=3))
        gpool = ctx.enter_context(tc.tile_pool(name="gates", bufs=3, space="PSUM"))
        ppool = ctx.enter_context(tc.tile_pool(name="ppsum", bufs=1, space="PSUM"))

        # ---- constants to SBUF
        whhT_p = consts.tile([H, 4 * H], BF16, tag="whhT_p")
        whhT_h = consts.tile([H, 4 * H], BF16, tag="whhT_h")
        lp = consts.tile([2, 4 * H], BF16, tag="lp")
        lh = consts.tile([1, 4 * H], BF16, tag="lh")
        woutZ = consts.tile([H, 2 * H], BF16, tag="woutZ")
        xq = consts.tile([2, NP * BS], BF16, tag="xq")
        ones = consts.tile([1, BS], BF16, tag="ones")
        nc.vector.memset(ones, 1.0)
        nc.sync.dma_start(out=whhT_p, in_=d["whhT_p"])
        nc.sync.dma_start(out=whhT_h, in_=d["whhT_h"])
        nc.sync.dma_start(out=lp, in_=d["lp"])
        nc.sync.dma_start(out=lh, in_=d["lh"])
        nc.sync.dma_start(out=woutZ, in_=d["woutZ"])
        nc.sync.dma_start(out=xq, in_=d["xq"])

        # ---- state: h kept in a 4-slot ring (slot s%4) so predictions can
        # be batched 4 steps per matmul against consecutive slots.
        hist = []
        cT = []
        for ch in range(NCHUNK):
            hh = state.tile([H, 4 * BC], BF16, tag=f"hist{ch}")
            c = state.tile([H, BC], BF16, tag=f"cT{ch}")
            nc.vector.memset(hh, 0.0)
            nc.vector.memset(c, 0.0)
            hist.append(hh)
            cT.append(c)

        pps = [None, None]
        sigs = [None, None]
        tgs = [None, None]
        gates_l = [None, None]

        def front(s, ch):
            """Gate matmuls + sigmoid/tanh activations for step s."""
            phase_p = s < NP
            gates = gpool.tile([H, 4 * H], F32, tag=f"g{ch}",
                               name=f"g{ch}_{s}")
            gates_l[ch] = gates
            whh = whhT_p if phase_p else whhT_h
            hprev = hist[ch][:, ((s - 1) % 4) * BC: ((s - 1) % 4 + 1) * BC]

            def gate_mm(k):
                go = gates[:, k * H:(k + 1) * H]
                nc.tensor.matmul(go, whh[:, k * H:(k + 1) * H], hprev,
                                 start=True, stop=False)
                if phase_p:
                    rhs = xq[0:2, s * BS + ch * BC: s * BS + ch * BC + BC]
                    lhs2 = lp[0:2, k * H:(k + 1) * H]
                else:
                    rhs = ones[0:1, ch * BC: ch * BC + BC]
                    lhs2 = lh[0:1, k * H:(k + 1) * H]
                nc.tensor.matmul(go, lhs2, rhs, start=False, stop=True)

            # g-gate first so tanh(g) can run on ACT while i/f/o matmuls
            # are still streaming; sigmoid follows.
            gate_mm(3)
            tg = work.tile([H, BC], BF16, tag=f"tg{ch}", name=f"tg{ch}_{s}")
            nc.scalar.activation(tg, gates[:, 3 * H:4 * H], AF.Tanh)
            for k in (0, 1, 2):
                gate_mm(k)
            sig = work.tile([H, 3 * H], BF16, tag=f"sig{ch}",
                            name=f"sig{ch}_{s}")
            nc.scalar.activation(sig, gates[:, 0:3 * H], AF.Sigmoid)
            sigs[ch] = sig
            tgs[ch] = tg

        def back(s, ch):
            """c/h update for step s + batched prediction matmul."""
            sig, tg = sigs[ch], tgs[ch]
            t2 = work.tile([H, BC], BF16, tag=f"t2{ch}", name=f"t2{ch}_{s}")
            nc.vector.tensor_mul(t2, sig[:, H:2 * H], cT[ch])
            t1 = work.tile([H, BC], BF16, tag=f"t1{ch}", name=f"t1{ch}_{s}")
            nc.gpsimd.tensor_mul(t1, sig[:, 0:H], tg)
            nc.vector.tensor_add(cT[ch], t2, t1)
            tcn = work.tile([H, BC], BF16, tag=f"tcn{ch}", name=f"tcn{ch}_{s}")
            nc.scalar.activation(tcn, cT[ch], AF.Tanh)
            hslot = hist[ch][:, (s % 4) * BC: (s % 4 + 1) * BC]
            nc.vector.tensor_mul(hslot, sig[:, 2 * H:3 * H], tcn)

            # Predictions: every 4 steps, p for steps 4G..4G+3 = one matmul
            # W_out @ [h_0|h_1|h_2|h_3]; row placement via shifted zero-pad.
            if s % 4 == 3 or s == NT - 1:
                G = s // 4
                r = G % 32
                n = (s % 4 + 1) * BC
                if r == 0:
                    pps[ch] = ppool.tile([H, 4 * BC], F32, tag=f"pps{ch}",
                                         name=f"pps{ch}_{s}")
                nc.tensor.matmul(pps[ch][:, 0:n],
                                 woutZ[:, H - r: 2 * H - r],
                                 hist[ch][:, 0:n],
                                 start=(r == 0), stop=(r == 31 or s == NT - 1),
                                 skip_group_check=True)
                if r == 31 or s == NT - 1:
                    e = G // 32
                    pc = work.tile([32, 4 * BC], F32, tag=f"pc{ch}",
                                   name=f"pc{ch}_{s}")
                    nc.vector.tensor_copy(pc, pps[ch][0:32, :])
                    nc.sync.dma_start(out=d["preds"][e, ch], in_=pc)

        # Software pipeline: chunk 1 runs half a step behind chunk 0 so
        # engines ping-pong between the two independent recurrences.
        for s in range(NT):
            front(s, 0)
            if s > 0:
                back(s - 1, 1)
            front(s, 1)
            back(s, 0)
        back(NT - 1, 1)


@functools.lru_cache(maxsize=2)
def _program(NP, NH, EPOCH):
    nc = bacc.Bacc("TRN2", target_bir_lowering=False, debug=False,
                   num_devices=NCORES)
    NT = NP + NH
    NEP = (NT + 127) // 128
    d = {
        "whhT_p": nc.dram_tensor("whhT_p", [H, 4 * H], BF16,
                                 kind="ExternalInput").ap(),
        "whhT_h": nc.dram_tensor("whhT_h", [H, 4 * H], BF16,
                                 kind="ExternalInput").ap(),
        "lp": nc.dram_tensor("lp", [2, 4 * H], BF16, kind="ExternalInput").ap(),
        "lh": nc.dram_tensor("lh", [1, 4 * H], BF16, kind="ExternalInput").ap(),
        "woutZ": nc.dram_tensor("woutZ", [H, 2 * H], BF16,
                                kind="ExternalInput").ap(),
        "xq": nc.dram_tensor("xq", [2, NP * BS], BF16,
                             kind="ExternalInput").ap(),
        "preds": nc.dram_tensor("preds", [NEP, NCHUNK, 32, 4 * BC], F32,
                                kind="ExternalOutput").ap(),
    }
    with tile.TileContext(nc) as tc:
        _build_body(tc, d, NP, NH, EPOCH)
    nc.compile()
    return nc


def _host_prep(y_flow, W_ih, W_hh, b_ih, b_hh, W_out, b_out, NP):
    """Build per-core input maps. y_flow: (B, T, 1) f32."""
    bf = ml_dtypes.bfloat16
    W_ih = np.asarray(W_ih, np.float32)
    W_hh = np.asarray(W_hh, np.float32)
    W_out = np.asarray(W_out, np.float32)
    bias = np.asarray(b_ih, np.float32) + np.asarray(b_hh, np.float32)
    b_out = np.asarray(b_out, np.float32)

    W_hh_H = W_hh + W_ih @ W_out          # [4H, H]
    bias_H = bias + W_ih[:, 0] * b_out[0]

    whhT_p = np.ascontiguousarray(W_hh[_PERM].T).astype(bf)      # [H, 4H]
    whhT_h = np.ascontiguousarray(W_hh_H[_PERM].T).astype(bf)
    lp = np.stack([W_ih[_PERM, 0], bias[_PERM]]).astype(bf)       # [2, 4H]
    lh = bias_H[_PERM][None, :].astype(bf)                        # [1, 4H]
    woutZ = np.zeros((H, 2 * H), np.float32)                      # [H, 256]
    woutZ[:, H] = W_out[0]
    woutZ = woutZ.astype(bf)

    y = np.asarray(y_flow, np.float32)[:, :, 0]                   # [B, T]
    B = y.shape[0]
    in_maps = []
    for core in range(NCORES):
        yc = y[core * BS:(core + 1) * BS]                         # [BS, T]
        xq = np.ones((2, NP * BS), np.float32)
        xq[0] = yc[:, :NP].T.reshape(-1)
        in_maps.append({
            "whhT_p": whhT_p, "whhT_h": whhT_h, "lp": lp, "lh": lh,
            "woutZ": woutZ, "xq": xq.astype(bf),
        })
    return in_maps


def kernel(y_flow, x_dyn, W_ih, W_hh, b_ih, b_hh, W_out, b_out, twin_idx,
           _trace=False):
    twin = int(twin_idx)
    assert twin == 256, f"kernel hardcodes twin_idx=256, got {twin}"
    B, T, _ = y_flow.shape
    assert (B, T) == (2048, 512)
    NP, NH, EPOCH = twin - 1, T - twin, 128
    NT = NP + NH

    nc = _program(NP, NH, EPOCH)
    in_maps = _host_prep(y_flow, W_ih, W_hh, b_ih, b_hh, W_out, b_out, NP)
    res = run_bass_kernel_spmd(nc, in_maps, core_ids=list(range(NCORES)),
                               trace=_trace)

    b_out = np.asarray(b_out, np.float32)
    out = np.empty((B, NT, 1), np.float32)
    for core in range(NCORES):
        p = np.asarray(res.results[core]["preds"], np.float32)
        nep = p.shape[0]
        a = p.reshape(nep, NCHUNK, 32, 4, BC)      # [e, ch, r, j, b]
        for ch in range(NCHUNK):
            blk = a[:, ch].transpose(3, 0, 1, 2).reshape(BC, -1)[:, :NT]
            out[core * BS + ch * BC: core * BS + (ch + 1) * BC, :, 0] = \
                blk + b_out[0]
    if _trace:
        kernel._last_results = res
    return out



# revision 3
# speedup vs baseline: 1.9712x; 1.9712x over previous
"""Bass/Trainium2 kernel for nn_BaselineLSTM (B=2048, T=512, H=128, twin=256).

Strategy (v2 — single-tanh gates, K=8 input matmul):
  - Data-parallel: batch 2048 -> 8 cores x 256; each core runs 2 interleaved
    chunks of 128 batch (pipelining hides per-step cross-engine latency).
  - State kept transposed: hist/cT = [H=128 partitions, batch free].
  - ALL FOUR gate activations are ONE tanh over the [128, 512] PSUM tile:
    sigmoid(x) = (tanh(x/2)+1)/2, so i/f/o rows are pre-scaled by 0.5 on
    the host. States carry a factor 2 (hh = 2h, D = 2c) so the affine
    corrections fold into fused scalar_tensor_tensor ops and the weights:
        u  = (tau_i + 1) * tau_g            [gpsimd stt]  = 2 sig_i * tanh(g)
        w  = (tau_f + 1) * D                [DVE stt]     = 4 sig_f * c
        D' = 0.5*w + u                      [DVE stt]     = 2 c'
        tcn = tanh(0.5 * D')                [ACT]         = tanh(c')
        hh = (tau_o + 1) * tcn              [DVE stt]     = 2 h'
    Matmul weights absorb the 1/2 of hh (and the i/f/o 0.5 pre-scale).
  - Input + bias enter via ONE K=8 matmul per chunk-step:
    lhsT = [alpha*W_ih rows stacked; alpha*bias rows stacked] [8, 128],
    rhs[r, k*128+b] = delta(k==r) * y_t[b] (r<4) / delta(k==r-4) (r>=4),
    host-precomputed per 32-step block, DMA-streamed (bufs=2 per chunk).
    Phase H: bias-only via constant K=4 one-hot rhs.
  - Phase H (autoregressive) feedback folded into the recurrence:
    W_hh_H = W_hh + W_ih W_out, bias_H = bias + W_ih b_out.
  - hh lives in a 4-slot SBUF ring; predictions p_t = (W_out/2) hh_t are
    batched 4 steps per matmul (shifted zero-padded stationary), flushed to
    DRAM every 128 steps. b_out is added on the host.
  - Two chunks emitted half a step out of phase (software pipeline).
    bf16 everywhere off-PSUM; PSUM accumulation f32.
"""

import functools

import ml_dtypes
import numpy as np

import concourse.bacc as bacc
import concourse.tile as tile
from concourse import mybir
from concourse.bass_utils import run_bass_kernel_spmd

F32 = mybir.dt.float32
BF16 = mybir.dt.bfloat16
AF = mybir.ActivationFunctionType
ALU = mybir.AluOpType

H = 128          # hidden
NCORES = 8
BS = 256         # batch per core
BC = 128         # batch per chunk
NCHUNK = 2
BLK = 32         # phase-P input steps per DMA block

# pytorch gate order (i, f, g, o) -> kernel order (i, f, o, g)
_PERM = np.concatenate([np.arange(0, 128), np.arange(128, 256),
                        np.arange(384, 512), np.arange(256, 384)])
# pre-tanh scale per kernel-order gate (sigmoid-via-tanh for i, f, o)
_ALPHA = np.array([0.5, 0.5, 0.5, 1.0], np.float32)

SI = slice(0 * H, 1 * H)   # tau slices (kernel gate order i|f|o|g)
SF = slice(1 * H, 2 * H)
SO = slice(2 * H, 3 * H)
SG = slice(3 * H, 4 * H)


def _build_body(tc, d, NP, NH, EPOCH):
    nc = tc.nc
    NT = NP + NH
    NBLK = (NP + BLK - 1) // BLK

    import contextlib
    with contextlib.ExitStack() as ctx:
        consts = ctx.enter_context(tc.tile_pool(name="consts", bufs=1))
        ypool = ctx.enter_context(tc.tile_pool(name="ypool", bufs=2))
        state = ctx.enter_context(tc.tile_pool(name="state", bufs=1))
        work = ctx.enter_context(tc.tile_pool(name="work", bufs=3))
        gpool = ctx.enter_context(tc.tile_pool(name="gates", bufs=3, space="PSUM"))
        ppool = ctx.enter_context(tc.tile_pool(name="ppsum", bufs=1, space="PSUM"))

        # ---- constants to SBUF
        whhT_p = consts.tile([H, 4 * H], BF16, tag="whhT_p")
        whhT_h = consts.tile([H, 4 * H], BF16, tag="whhT_h")
        lpK8 = consts.tile([8, H], BF16, tag="lpK8")
        lhK4 = consts.tile([4, H], BF16, tag="lhK4")
        onehot4 = consts.tile([4, 4 * H], BF16, tag="onehot4")
        woutZ = consts.tile([H, 2 * H], BF16, tag="woutZ")
        nc.sync.dma_start(out=whhT_p, in_=d["whhT_p"])
        nc.sync.dma_start(out=whhT_h, in_=d["whhT_h"])
        nc.sync.dma_start(out=lpK8, in_=d["lpK8"])
        nc.sync.dma_start(out=lhK4, in_=d["lhK4"])
        nc.sync.dma_start(out=onehot4, in_=d["onehot4"])
        nc.sync.dma_start(out=woutZ, in_=d["woutZ"])

        # ---- phase-P input blocks, streamed (bufs=2 per chunk tag)
        ybt = {}

        def dma_yblk(b):
            for ch in range(NCHUNK):
                t = ypool.tile([8, BLK * 4 * H], BF16, tag=f"yb{ch}",
                               name=f"yb{ch}_{b}")
                nc.sync.dma_start(out=t, in_=d["yblk"][ch, b])
                ybt[(ch, b)] = t

        dma_yblk(0)

        # ---- state: hh kept in a 4-slot ring (slot s%4) so predictions can
        # be batched 4 steps per matmul against consecutive slots.
        hist = []
        dT = []
        for ch in range(NCHUNK):
            hh = state.tile([H, 4 * BC], BF16, tag=f"hist{ch}")
            dd = state.tile([H, BC], BF16, tag=f"dT{ch}")
            nc.vector.memset(hh, 0.0)
            nc.vector.memset(dd, 0.0)
            hist.append(hh)
            dT.append(dd)

        pps = [None, None]
        taus = [None, None]

        def front(s, ch):
            """Input matmul + 4 gate matmuls + merged tanh for step s."""
            phase_p = s < NP
            gates = gpool.tile([H, 4 * H], F32, tag=f"g{ch}",
                               name=f"g{ch}_{s}")
            if phase_p:
                b, off = divmod(s, BLK)
                if off == 0 and ch == 0 and b + 1 < NBLK:
                    dma_yblk(b + 1)
                rhs = ybt[(ch, b)][:, off * 4 * H:(off + 1) * 4 * H]
                nc.tensor.matmul(gates, lpK8, rhs, start=True, stop=False,
                                 skip_group_check=True)
            else:
                nc.tensor.matmul(gates, lhK4, onehot4, start=True, stop=False,
                                 skip_group_check=True)
            whh = whhT_p if phase_p else whhT_h
            hprev = hist[ch][:, ((s - 1) % 4) * BC: ((s - 1) % 4 + 1) * BC]
            for k in range(4):
                nc.tensor.matmul(gates[:, k * H:(k + 1) * H],
                                 whh[:, k * H:(k + 1) * H], hprev,
                                 start=False, stop=(k == 3),
                                 skip_group_check=True)
            tau = work.tile([H, 4 * H], BF16, tag=f"tau{ch}",
                            name=f"tau{ch}_{s}")
            nc.scalar.activation(tau, gates, AF.Tanh)
            taus[ch] = tau

        def back(s, ch):
            """c/h update for step s + batched prediction matmul."""
            tau = taus[ch]
            u = work.tile([H, BC], BF16, tag=f"u{ch}", name=f"u{ch}_{s}")
            nc.vector.scalar_tensor_tensor(u, tau[:, SI], 1.0, tau[:, SG],
                                           ALU.add, ALU.mult)
            w = work.tile([H, BC], BF16, tag=f"w{ch}", name=f"w{ch}_{s}")
            nc.vector.scalar_tensor_tensor(w, tau[:, SF], 1.0, dT[ch],
                                           ALU.add, ALU.mult)
            nc.vector.scalar_tensor_tensor(dT[ch], w, 0.5, u,
                                           ALU.mult, ALU.add)
            tcn = work.tile([H, BC], BF16, tag=f"tcn{ch}", name=f"tcn{ch}_{s}")
            nc.scalar.activation(tcn, dT[ch], AF.Tanh, scale=0.5)
            hslot = hist[ch][:, (s % 4) * BC: (s % 4 + 1) * BC]
            nc.vector.scalar_tensor_tensor(hslot, tau[:, SO], 1.0, tcn,
                                           ALU.add, ALU.mult)

            # Predictions: every 4 steps, p for steps 4G..4G+3 = one matmul
            # (W_out/2) @ [hh_0|..|hh_3]; row placement via shifted zero-pad.
            if s % 4 == 3 or s == NT - 1:
                G = s // 4
                r = G % 32
                n = (s % 4 + 1) * BC
                if r == 0:
                    pps[ch] = ppool.tile([H, 4 * BC], F32, tag=f"pps{ch}",
                                         name=f"pps{ch}_{s}")
                nc.tensor.matmul(pps[ch][:, 0:n],
                                 woutZ[:, H - r: 2 * H - r],
                                 hist[ch][:, 0:n],
                                 start=(r == 0), stop=(r == 31 or s == NT - 1),
                                 skip_group_check=True)
                if r == 31 or s == NT - 1:
                    e = G // 32
                    pc = work.tile([32, 4 * BC], F32, tag=f"pc{ch}",
                                   name=f"pc{ch}_{s}")
                    nc.vector.tensor_copy(pc, pps[ch][0:32, :])
                    nc.sync.dma_start(out=d["preds"][e, ch], in_=pc)

        # Software pipeline: chunk 1 runs half a step behind chunk 0 so
        # engines ping-pong between the two independent recurrences.
        for s in range(NT):
            front(s, 0)
            if s > 0:
                back(s - 1, 1)
            front(s, 1)
            back(s, 0)
        back(NT - 1, 1)


@functools.lru_cache(maxsize=2)
def _program(NP, NH, EPOCH):
    nc = bacc.Bacc("TRN2", target_bir_lowering=False, debug=False,
                   num_devices=NCORES)
    NT = NP + NH
    NEP = (NT + 127) // 128
    NBLK = (NP + BLK - 1) // BLK
    d = {
        "whhT_p": nc.dram_tensor("whhT_p", [H, 4 * H], BF16,
                                 kind="ExternalInput").ap(),
        "whhT_h": nc.dram_tensor("whhT_h", [H, 4 * H], BF16,
                                 kind="ExternalInput").ap(),
        "lpK8": nc.dram_tensor("lpK8", [8, H], BF16,
                               kind="ExternalInput").ap(),
        "lhK4": nc.dram_tensor("lhK4", [4, H], BF16,
                               kind="ExternalInput").ap(),
        "onehot4": nc.dram_tensor("onehot4", [4, 4 * H], BF16,
                                  kind="ExternalInput").ap(),
        "woutZ": nc.dram_tensor("woutZ", [H, 2 * H], BF16,
                                kind="ExternalInput").ap(),
        "yblk": nc.dram_tensor("yblk", [NCHUNK, NBLK, 8, BLK * 4 * H], BF16,
                               kind="ExternalInput").ap(),
        "preds": nc.dram_tensor("preds", [NEP, NCHUNK, 32, 4 * BC], F32,
                                kind="ExternalOutput").ap(),
    }
    with tile.TileContext(nc) as tc:
        _build_body(tc, d, NP, NH, EPOCH)
    nc.compile()
    return nc


def _host_prep(y_flow, W_ih, W_hh, b_ih, b_hh, W_out, b_out, NP):
    """Build per-core input maps. y_flow: (B, T, 1) f32."""
    bf = ml_dtypes.bfloat16
    W_ih = np.asarray(W_ih, np.float32)
    W_hh = np.asarray(W_hh, np.float32)
    W_out = np.asarray(W_out, np.float32)
    bias = np.asarray(b_ih, np.float32) + np.asarray(b_hh, np.float32)
    b_out = np.asarray(b_out, np.float32)
    NBLK = (NP + BLK - 1) // BLK

    W_hh_H = W_hh + W_ih @ W_out          # [4H, H]
    bias_H = bias + W_ih[:, 0] * b_out[0]

    # per-row scale in kernel gate order: alpha_k (sigmoid-via-tanh) and the
    # extra 1/2 on recurrent weights because the state is hh = 2h.
    arow = np.repeat(_ALPHA, H)                                  # [4H]
    whhT_p = ((arow[:, None] * 0.5) * W_hh[_PERM]).T             # [H, 4H]
    whhT_h = ((arow[:, None] * 0.5) * W_hh_H[_PERM]).T
    whhT_p = np.ascontiguousarray(whhT_p).astype(bf)
    whhT_h = np.ascontiguousarray(whhT_h).astype(bf)

    lpK8 = np.zeros((8, H), np.float32)                          # [8, 128]
    lhK4 = np.zeros((4, H), np.float32)
    for k in range(4):
        rows = _PERM[k * H:(k + 1) * H]
        lpK8[k] = _ALPHA[k] * W_ih[rows, 0]
        lpK8[4 + k] = _ALPHA[k] * bias[rows]
        lhK4[k] = _ALPHA[k] * bias_H[rows]
    onehot4 = np.zeros((4, 4 * H), np.float32)
    for k in range(4):
        onehot4[k, k * H:(k + 1) * H] = 1.0

    woutZ = np.zeros((H, 2 * H), np.float32)                     # [H, 256]
    woutZ[:, H] = 0.5 * W_out[0]

    y = np.asarray(y_flow, np.float32)[:, :, 0]                  # [B, T]
    B = y.shape[0]
    in_maps = []
    common = {
        "whhT_p": whhT_p, "whhT_h": whhT_h,
        "lpK8": lpK8.astype(bf), "lhK4": lhK4.astype(bf),
        "onehot4": onehot4.astype(bf), "woutZ": woutZ.astype(bf),
    }
    NSP = NBLK * BLK
    for core in range(NCORES):
        yblk = np.zeros((NCHUNK, 8, NSP, 4, H), np.float32)
        for ch in range(NCHUNK):
            yc = y[core * BS + ch * BC: core * BS + (ch + 1) * BC, :NP]
            for k in range(4):
                yblk[ch, k, :NP, k, :] = yc.T          # [NP, BC]
                yblk[ch, 4 + k, :, k, :] = 1.0
        yblk = yblk.reshape(NCHUNK, 8, NBLK, BLK * 4 * H)
        yblk = np.ascontiguousarray(yblk.transpose(0, 2, 1, 3))
        in_maps.append(dict(common, yblk=yblk.astype(bf)))
    return in_maps


def kernel(y_flow, x_dyn, W_ih, W_hh, b_ih, b_hh, W_out, b_out, twin_idx,
           _trace=False):
    twin = int(twin_idx)
    assert twin == 256, f"kernel hardcodes twin_idx=256, got {twin}"
    B, T, _ = y_flow.shape
    assert (B, T) == (2048, 512)
    NP, NH, EPOCH = twin - 1, T - twin, 128
    NT = NP + NH

    nc = _program(NP, NH, EPOCH)
    in_maps = _host_prep(y_flow, W_ih, W_hh, b_ih, b_hh, W_out, b_out, NP)
    res = run_bass_kernel_spmd(nc, in_maps, core_ids=list(range(NCORES)),
                               trace=_trace)

    b_out = np.asarray(b_out, np.float32)
    out = np.empty((B, NT, 1), np.float32)
    for core in range(NCORES):
        p = np.asarray(res.results[core]["preds"], np.float32)
        nep = p.shape[0]
        a = p.reshape(nep, NCHUNK, 32, 4, BC)      # [e, ch, r, j, b]
        for ch in range(NCHUNK):
            blk = a[:, ch].transpose(3, 0, 1, 2).reshape(BC, -1)[:, :NT]
            out[core * BS + ch * BC: core * BS + (ch + 1) * BC, :, 0] = \
                blk + b_out[0]
    if _trace:
        kernel._last_results = res
    return out


# revision 4
# speedup vs baseline: 2.1128x; 1.0718x over previous
"""Bass/Trainium2 kernel for nn_BaselineLSTM (B=2048, T=512, H=128, twin=256).

Strategy (v2 — single-tanh gates, K=8 input matmul):
  - Data-parallel: batch 2048 -> 8 cores x 256; each core runs 2 interleaved
    chunks of 128 batch (pipelining hides per-step cross-engine latency).
  - State kept transposed: hist/cT = [H=128 partitions, batch free].
  - ALL FOUR gate activations are ONE tanh over the [128, 512] PSUM tile:
    sigmoid(x) = (tanh(x/2)+1)/2, so i/f/o rows are pre-scaled by 0.5 on
    the host. States carry a factor 2 (hh = 2h, D = 2c) so the affine
    corrections fold into fused scalar_tensor_tensor ops and the weights:
        u  = (tau_i + 1) * tau_g            [gpsimd stt]  = 2 sig_i * tanh(g)
        w  = (tau_f + 1) * D                [DVE stt]     = 4 sig_f * c
        D' = 0.5*w + u                      [DVE stt]     = 2 c'
        tcn = tanh(0.5 * D')                [ACT]         = tanh(c')
        hh = (tau_o + 1) * tcn              [DVE stt]     = 2 h'
    Matmul weights absorb the 1/2 of hh (and the i/f/o 0.5 pre-scale).
  - Input + bias enter via ONE K=8 matmul per chunk-step:
    lhsT = [alpha*W_ih rows stacked; alpha*bias rows stacked] [8, 128],
    rhs[r, k*128+b] = delta(k==r) * y_t[b] (r<4) / delta(k==r-4) (r>=4),
    host-precomputed per 32-step block, DMA-streamed (bufs=2 per chunk).
    Phase H: bias-only via constant K=4 one-hot rhs.
  - Phase H (autoregressive) feedback folded into the recurrence:
    W_hh_H = W_hh + W_ih W_out, bias_H = bias + W_ih b_out.
  - hh lives in a 4-slot SBUF ring; predictions p_t = (W_out/2) hh_t are
    batched 4 steps per matmul (shifted zero-padded stationary), flushed to
    DRAM every 128 steps. b_out is added on the host.
  - Two chunks emitted half a step out of phase (software pipeline).
    bf16 everywhere off-PSUM; PSUM accumulation f32.
"""

import functools

import ml_dtypes
import numpy as np

import concourse.bacc as bacc
import concourse.tile as tile
from concourse import mybir
from concourse.bass_utils import run_bass_kernel_spmd

F32 = mybir.dt.float32
BF16 = mybir.dt.bfloat16
AF = mybir.ActivationFunctionType
ALU = mybir.AluOpType

H = 128          # hidden
NCORES = 8
BS = 256         # batch per core
BC = 128         # batch per chunk
NCHUNK = 2
BLK = 32         # phase-P input steps per DMA block

# pytorch gate order (i, f, g, o) -> kernel order (i, f, o, g)
_PERM = np.concatenate([np.arange(0, 128), np.arange(128, 256),
                        np.arange(384, 512), np.arange(256, 384)])
# pre-tanh scale per kernel-order gate (sigmoid-via-tanh for i, f, o)
_ALPHA = np.array([0.5, 0.5, 0.5, 1.0], np.float32)

SI = slice(0 * H, 1 * H)   # tau slices (kernel gate order i|f|o|g)
SF = slice(1 * H, 2 * H)
SO = slice(2 * H, 3 * H)
SG = slice(3 * H, 4 * H)


def _build_body(tc, d, NP, NH, EPOCH):
    nc = tc.nc
    NT = NP + NH
    NBLK = (NP + BLK - 1) // BLK

    import contextlib
    with contextlib.ExitStack() as ctx:
        consts = ctx.enter_context(tc.tile_pool(name="consts", bufs=1))
        ypool = ctx.enter_context(tc.tile_pool(name="ypool", bufs=2))
        state = ctx.enter_context(tc.tile_pool(name="state", bufs=1))
        work = ctx.enter_context(tc.tile_pool(name="work", bufs=3))
        gpool = ctx.enter_context(tc.tile_pool(name="gates", bufs=3, space="PSUM"))
        ppool = ctx.enter_context(tc.tile_pool(name="ppsum", bufs=1, space="PSUM"))

        # ---- constants to SBUF
        whhT_p = consts.tile([H, 4 * H], BF16, tag="whhT_p")
        whhT_h = consts.tile([H, 4 * H], BF16, tag="whhT_h")
        lpK8 = consts.tile([8, H], BF16, tag="lpK8")
        lhK4 = consts.tile([4, H], BF16, tag="lhK4")
        onehot4 = consts.tile([4, 4 * H], BF16, tag="onehot4")
        woutZ = consts.tile([H, 2 * H], BF16, tag="woutZ")
        nc.sync.dma_start(out=whhT_p, in_=d["whhT_p"])
        nc.sync.dma_start(out=whhT_h, in_=d["whhT_h"])
        nc.sync.dma_start(out=lpK8, in_=d["lpK8"])
        nc.sync.dma_start(out=lhK4, in_=d["lhK4"])
        nc.sync.dma_start(out=onehot4, in_=d["onehot4"])
        nc.sync.dma_start(out=woutZ, in_=d["woutZ"])

        # ---- phase-P input blocks, streamed (bufs=2 per chunk tag)
        ybt = {}

        def dma_yblk(b):
            for ch in range(NCHUNK):
                t = ypool.tile([8, BLK * 4 * H], BF16, tag=f"yb{ch}",
                               name=f"yb{ch}_{b}")
                nc.sync.dma_start(out=t, in_=d["yblk"][ch, b])
                ybt[(ch, b)] = t

        dma_yblk(0)

        # ---- state: hh kept in a 4-slot ring (slot s%4) so predictions can
        # be batched 4 steps per matmul against consecutive slots.
        hist = []
        dT = []
        for ch in range(NCHUNK):
            hh = state.tile([H, 4 * BC], BF16, tag=f"hist{ch}")
            dd = state.tile([H, BC], BF16, tag=f"dT{ch}")
            nc.vector.memset(hh, 0.0)
            nc.vector.memset(dd, 0.0)
            hist.append(hh)
            dT.append(dd)

        pps = [None, None]
        taus = [None, None]
        # tau tile layout: [i|f|o|g|D] (640 cols). ACT writes 0:512; the
        # PREVIOUS step's D' stt wrote D=2c into 512:640, so
        # u|w = (tau[0:256] + 1) * tau[384:640] is ONE fused DVE op.
        taus_next = []
        for ch in range(NCHUNK):
            t = work.tile([H, 4 * H + BC], BF16, tag=f"tau{ch}",
                          name=f"tau{ch}_init")
            nc.vector.memset(t[:, 4 * H:], 0.0)
            taus_next.append(t)

        def front(s, ch):
            """Input matmul + 4 gate matmuls + merged tanh for step s."""
            phase_p = s < NP
            gates = gpool.tile([H, 4 * H], F32, tag=f"g{ch}",
                               name=f"g{ch}_{s}")
            if phase_p:
                b, off = divmod(s, BLK)
                if off == 0 and ch == 0 and b + 1 < NBLK:
                    dma_yblk(b + 1)
                rhs = ybt[(ch, b)][:, off * 4 * H:(off + 1) * 4 * H]
                nc.tensor.matmul(gates, lpK8, rhs, start=True, stop=False,
                                 skip_group_check=True)
            else:
                nc.tensor.matmul(gates, lhK4, onehot4, start=True, stop=False,
                                 skip_group_check=True)
            whh = whhT_p if phase_p else whhT_h
            hprev = hist[ch][:, ((s - 1) % 4) * BC: ((s - 1) % 4 + 1) * BC]
            for k in range(4):
                nc.tensor.matmul(gates[:, k * H:(k + 1) * H],
                                 whh[:, k * H:(k + 1) * H], hprev,
                                 start=False, stop=(k == 3),
                                 skip_group_check=True)
            tau = taus_next[ch]
            nc.scalar.activation(tau[:, 0:4 * H], gates, AF.Tanh)
            taus[ch] = tau

        def back(s, ch):
            """c/h update for step s."""
            tau = taus[ch]
            uw = work.tile([H, 2 * BC], BF16, tag=f"uw{ch}", name=f"uw{ch}_{s}")
            nc.vector.scalar_tensor_tensor(uw, tau[:, 0:2 * H], 1.0,
                                           tau[:, 3 * H:3 * H + 2 * BC],
                                           ALU.add, ALU.mult)
            taun = work.tile([H, 4 * H + BC], BF16, tag=f"tau{ch}",
                             name=f"tau{ch}_{s + 1}")
            nc.vector.scalar_tensor_tensor(taun[:, 4 * H:], uw[:, BC:], 0.5,
                                           uw[:, 0:BC], ALU.mult, ALU.add)
            tcn = work.tile([H, BC], BF16, tag=f"tcn{ch}", name=f"tcn{ch}_{s}")
            nc.scalar.activation(tcn, taun[:, 4 * H:], AF.Tanh, scale=0.5)
            hslot = hist[ch][:, (s % 4) * BC: (s % 4 + 1) * BC]
            nc.vector.scalar_tensor_tensor(hslot, tau[:, SO], 1.0, tcn,
                                           ALU.add, ALU.mult)
            taus_next[ch] = taun

        def pred(s, ch):
            """Batched prediction matmul for steps 4G..s (emitted one step
            late so it never blocks the gate matmuls in the PE queue)."""
            if s < 0 or not (s % 4 == 3 or s == NT - 1):
                return
            G = s // 4
            r = G % 32
            n = (s % 4 + 1) * BC
            if r == 0:
                pps[ch] = ppool.tile([H, 4 * BC], F32, tag=f"pps{ch}",
                                     name=f"pps{ch}_{s}")
            nc.tensor.matmul(pps[ch][:, 0:n],
                             woutZ[:, H - r: 2 * H - r],
                             hist[ch][:, 0:n],
                             start=(r == 0), stop=(r == 31 or s == NT - 1),
                             skip_group_check=True)
            if r == 31 or s == NT - 1:
                e = G // 32
                pc = work.tile([32, 4 * BC], F32, tag=f"pc{ch}",
                               name=f"pc{ch}_{s}")
                nc.vector.tensor_copy(pc, pps[ch][0:32, :])
                nc.sync.dma_start(out=d["preds"][e, ch], in_=pc)

        # Software pipeline: chunk 1 runs half a step behind chunk 0 so
        # engines ping-pong between the two independent recurrences.
        for s in range(NT):
            front(s, 0)
            pred(s - 1, 0)
            if s > 0:
                back(s - 1, 1)
            front(s, 1)
            pred(s - 1, 1)
            back(s, 0)
        back(NT - 1, 1)
        pred(NT - 1, 0)
        pred(NT - 1, 1)


@functools.lru_cache(maxsize=2)
def _program(NP, NH, EPOCH):
    nc = bacc.Bacc("TRN2", target_bir_lowering=False, debug=False,
                   num_devices=NCORES)
    NT = NP + NH
    NEP = (NT + 127) // 128
    NBLK = (NP + BLK - 1) // BLK
    d = {
        "whhT_p": nc.dram_tensor("whhT_p", [H, 4 * H], BF16,
                                 kind="ExternalInput").ap(),
        "whhT_h": nc.dram_tensor("whhT_h", [H, 4 * H], BF16,
                                 kind="ExternalInput").ap(),
        "lpK8": nc.dram_tensor("lpK8", [8, H], BF16,
                               kind="ExternalInput").ap(),
        "lhK4": nc.dram_tensor("lhK4", [4, H], BF16,
                               kind="ExternalInput").ap(),
        "onehot4": nc.dram_tensor("onehot4", [4, 4 * H], BF16,
                                  kind="ExternalInput").ap(),
        "woutZ": nc.dram_tensor("woutZ", [H, 2 * H], BF16,
                                kind="ExternalInput").ap(),
        "yblk": nc.dram_tensor("yblk", [NCHUNK, NBLK, 8, BLK * 4 * H], BF16,
                               kind="ExternalInput").ap(),
        "preds": nc.dram_tensor("preds", [NEP, NCHUNK, 32, 4 * BC], F32,
                                kind="ExternalOutput").ap(),
    }
    with tile.TileContext(nc) as tc:
        _build_body(tc, d, NP, NH, EPOCH)
    nc.compile()
    return nc


def _host_prep(y_flow, W_ih, W_hh, b_ih, b_hh, W_out, b_out, NP):
    """Build per-core input maps. y_flow: (B, T, 1) f32."""
    bf = ml_dtypes.bfloat16
    W_ih = np.asarray(W_ih, np.float32)
    W_hh = np.asarray(W_hh, np.float32)
    W_out = np.asarray(W_out, np.float32)
    bias = np.asarray(b_ih, np.float32) + np.asarray(b_hh, np.float32)
    b_out = np.asarray(b_out, np.float32)
    NBLK = (NP + BLK - 1) // BLK

    W_hh_H = W_hh + W_ih @ W_out          # [4H, H]
    bias_H = bias + W_ih[:, 0] * b_out[0]

    # per-row scale in kernel gate order: alpha_k (sigmoid-via-tanh) and the
    # extra 1/2 on recurrent weights because the state is hh = 2h.
    arow = np.repeat(_ALPHA, H)                                  # [4H]
    whhT_p = ((arow[:, None] * 0.5) * W_hh[_PERM]).T             # [H, 4H]
    whhT_h = ((arow[:, None] * 0.5) * W_hh_H[_PERM]).T
    whhT_p = np.ascontiguousarray(whhT_p).astype(bf)
    whhT_h = np.ascontiguousarray(whhT_h).astype(bf)

    lpK8 = np.zeros((8, H), np.float32)                          # [8, 128]
    lhK4 = np.zeros((4, H), np.float32)
    for k in range(4):
        rows = _PERM[k * H:(k + 1) * H]
        lpK8[k] = _ALPHA[k] * W_ih[rows, 0]
        lpK8[4 + k] = _ALPHA[k] * bias[rows]
        lhK4[k] = _ALPHA[k] * bias_H[rows]
    onehot4 = np.zeros((4, 4 * H), np.float32)
    for k in range(4):
        onehot4[k, k * H:(k + 1) * H] = 1.0

    woutZ = np.zeros((H, 2 * H), np.float32)                     # [H, 256]
    woutZ[:, H] = 0.5 * W_out[0]

    y = np.asarray(y_flow, np.float32)[:, :, 0]                  # [B, T]
    B = y.shape[0]
    in_maps = []
    common = {
        "whhT_p": whhT_p, "whhT_h": whhT_h,
        "lpK8": lpK8.astype(bf), "lhK4": lhK4.astype(bf),
        "onehot4": onehot4.astype(bf), "woutZ": woutZ.astype(bf),
    }
    NSP = NBLK * BLK
    for core in range(NCORES):
        yblk = np.zeros((NCHUNK, 8, NSP, 4, H), np.float32)
        for ch in range(NCHUNK):
            yc = y[core * BS + ch * BC: core * BS + (ch + 1) * BC, :NP]
            for k in range(4):
                yblk[ch, k, :NP, k, :] = yc.T          # [NP, BC]
                yblk[ch, 4 + k, :, k, :] = 1.0
        yblk = yblk.reshape(NCHUNK, 8, NBLK, BLK * 4 * H)
        yblk = np.ascontiguousarray(yblk.transpose(0, 2, 1, 3))
        in_maps.append(dict(common, yblk=yblk.astype(bf)))
    return in_maps


def kernel(y_flow, x_dyn, W_ih, W_hh, b_ih, b_hh, W_out, b_out, twin_idx,
           _trace=False):
    twin = int(twin_idx)
    assert twin == 256, f"kernel hardcodes twin_idx=256, got {twin}"
    B, T, _ = y_flow.shape
    assert (B, T) == (2048, 512)
    NP, NH, EPOCH = twin - 1, T - twin, 128
    NT = NP + NH

    nc = _program(NP, NH, EPOCH)
    in_maps = _host_prep(y_flow, W_ih, W_hh, b_ih, b_hh, W_out, b_out, NP)
    res = run_bass_kernel_spmd(nc, in_maps, core_ids=list(range(NCORES)),
                               trace=_trace)

    b_out = np.asarray(b_out, np.float32)
    out = np.empty((B, NT, 1), np.float32)
    for core in range(NCORES):
        p = np.asarray(res.results[core]["preds"], np.float32)
        nep = p.shape[0]
        a = p.reshape(nep, NCHUNK, 32, 4, BC)      # [e, ch, r, j, b]
        for ch in range(NCHUNK):
            blk = a[:, ch].transpose(3, 0, 1, 2).reshape(BC, -1)[:, :NT]
            out[core * BS + ch * BC: core * BS + (ch + 1) * BC, :, 0] = \
                blk + b_out[0]
    if _trace:
        kernel._last_results = res
    return out
